# revision 29
# baseline (speedup 1.0000x reference)
"""Trainium2 Bass kernel for DensePairwiseRelaxedWordMoverSimilarity.

Shapes (hardcoded): x1 (64,128,512) f32, mask1 (64,128) bool,
                    x2 (64,128,512) f32, mask2 (64,128) bool -> out (64,64) f32.

Strategy: data-parallel over B1 across 8 cores; core k holds x1 rows
[8k, 8k+8) plus the full x2 and produces an (8, 64) output slab.

v3: single-orientation fp8 matmul + exp-domain reductions.
For each (a, 4-b chunk) the PE computes C^T tiles [128 s, 4b x 128 t]
(fp8 DoubleRow, f32 PSUM).  The scalar engine applies
E = exp(beta*(C - c0)) into bf16 SBUF (trimmed to the chunk's baked
t-extent).  Then BOTH reductions come cheap:
  - sim1[a,b,s] = max_t C = (ln max_t E)/beta + c0: segmented DVE
    reduce_max over the free dim (optionally gpsimd pre-halving),
  - sim2[a,b,t] via log-sum-exp: sum_s E is a PE matmul with an
    all-ones stationary column, accumulated per (chunk, a) into one
    PSUM bank row; masked-s rows contribute exp(-beta*c0) ~ 1e-38 (b
    columns are sorted+trimmed so masked-t never pollutes sums).
    The LSE overshoot at beta=250 measures ~3e-3 rel on this data,
    under the fp8 quantization floor.
One Ln pass per collection (+1e-36 bias so memset-zero pad columns
stay finite) and tiny w1/w2-weighted matmul/reduce means finish on
device; host adds the (m1+m2)/beta + c0 affine and unsorts b.
"""

import numpy as np
import ml_dtypes

import concourse.bacc as bacc
import concourse.mybir as mybir
from concourse import tile
from concourse.bass_utils import run_bass_kernel_spmd

F32 = mybir.dt.float32
BF16 = mybir.dt.bfloat16
FP8 = mybir.dt.float8e4
AX = mybir.AxisListType
AF = mybir.ActivationFunctionType
ALU = mybir.AluOpType
DR = mybir.MatmulPerfMode.DoubleRow

E4NP = ml_dtypes.float8_e4m3
BFNP = ml_dtypes.bfloat16

B1, S1, B2, S2, D = 64, 128, 64, 128, 512
NCORES = 8
A = B1 // NCORES          # 8 x1 rows per core
X1W = A * S1              # 1024 x1 columns
X2W = B2 * S2             # 8192 x2 columns
NCH = 16                  # chunks of 4 sorted b's (512 x2 columns)
QSCALE = 32.0             # fp8 pre-scale; sims carry QSCALE^2
BETA = 250.0              # LSE sharpness
C0 = 0.35                 # exp recentering; keeps E <= ~1
LNEPS = 1e-36             # (unused since bit-log; kept in consts layout)
DEFER = 3                 # units of lag for E consumers (PE slack)

_CACHE = {}


def _build(L2G):
    """L2G: 16 baked t-extents (even), one per sorted 4-b chunk."""
    nc = bacc.Bacc(None, target_bir_lowering=False, debug=False)

    x1p = nc.declare_dram_parameter("x1p", [D, X1W], FP8, isOutput=False)
    x2p = nc.declare_dram_parameter("x2p", [D, X2W], FP8, isOutput=False)
    consts = nc.declare_dram_parameter("consts", [128, 4], F32, isOutput=False)
    w2patp = nc.declare_dram_parameter("w2patp", [128, 512], F32, isOutput=False)
    w1Tp = nc.declare_dram_parameter("w1Tp", [128, A], F32, isOutput=False)
    m1o = nc.declare_dram_parameter("m1o", [A, A * B2], F32, isOutput=True)
    m2o = nc.declare_dram_parameter("m2o", [128, 4], F32, isOutput=True)

    exp_scale = float(BETA / (QSCALE * QSCALE))

    with tile.TileContext(nc) as tc:
        with (
            tc.tile_pool(name="xts", bufs=1) as xts_pool,
            tc.tile_pool(name="cst", bufs=1) as cst_pool,
            tc.tile_pool(name="coll", bufs=1) as coll_pool,
            tc.tile_pool(name="epool", bufs=6) as epool,
            tc.tile_pool(name="hpool", bufs=3) as hpool,
            tc.tile_pool(name="psS", bufs=3, space="PSUM") as psS,
            tc.tile_pool(name="psF", bufs=1, space="PSUM") as psF,
            tc.tile_pool(name="psM", bufs=1, space="PSUM") as psM,
        ):
            # ---- loads: first chunk + x1 first (they gate the first
            # matmul), then constants, then the remaining x2 blocks.
            x2c = [None] * NCH
            blocks = [(0, 1), (1, 1), (2, 2), (4, 4), (8, 8)]
            xb0 = xts_pool.tile([128, 4, 512], FP8, tag="xb0")
            nc.sync.dma_start(
                xb0[:],
                x2p.ap()[:, 0:512].rearrange("(k p) m -> p k m", p=128),
            )
            x2c[0] = xb0[:, :, :]
            x1t = xts_pool.tile([128, 4, X1W], FP8, tag="x1t")
            nc.sync.dma_start(
                x1t[:], x1p.ap().rearrange("(k p) m -> p k m", p=128)
            )
            csts = cst_pool.tile([128, 4], F32, tag="consts")
            nc.sync.dma_start(csts[:], consts.ap())
            for g0, w in blocks[1:]:
                xb = xts_pool.tile([128, 4, w * 512], FP8, tag=f"xb{g0}")
                nc.sync.dma_start(
                    xb[:],
                    x2p.ap()[:, g0 * 512 : (g0 + w) * 512].rearrange(
                        "(k p) m -> p k m", p=128
                    ),
                )
                for j in range(w):
                    x2c[g0 + j] = xb[:, :, j * 512 : (j + 1) * 512]
            w2pat = cst_pool.tile([128, 512], F32, tag="w2pat")
            nc.sync.dma_start(w2pat[:], w2patp.ap())
            w1T = cst_pool.tile([128, A], F32, tag="w1T")
            nc.sync.dma_start(w1T[:], w1Tp.ap())

            expbias = csts[:, 1:2]      # -BETA*C0
            # sliding-window one-hot: col 64 is all-ones, so the width-64
            # slice [64-m : 128-m] has its ones in column m.
            oh64 = cst_pool.tile([128, 128], BF16, tag="oh64")
            nc.vector.memset(oh64[:], 0.0)
            nc.vector.tensor_copy(
                oh64[:, 64:65], csts[:, 0:1]
            )

            # maxE collection [s, a, sorted-b]; SumE PSUM bank rows (8c+a)
            sim1st = coll_pool.tile([128, A, B2], BF16, tag="sim1st")
            sumE = psF.tile([128, 512], F32, tag="sumE")
            nc.vector.memset(sumE[:], 0.0)

            def emit_mm(u):
                """fp8 DoubleRow matmuls for unit u = (chunk, a-pair);
                moving columns trimmed to the chunk's baked t-extent."""
                c, ap_ = u // 4, (u % 4) * 2
                l = L2G[c]
                S = psS.tile([128, 2, 4, 128], F32, tag="S", name=f"S{u}")
                xv = x2c[c].rearrange("p k (g t) -> p k g t", g=4)
                for j in range(2):
                    a = ap_ + j
                    for i, (k0, k1) in enumerate(((0, 2), (2, 4))):
                        nc.tensor.matmul(
                            S[:, j, :, 0:l],
                            x1t[:, k0:k1, a * 128 : (a + 1) * 128],
                            xv[:, k0:k1, :, 0:l],
                            start=(i == 0),
                            stop=(i == 1),
                            perf_mode=DR,
                        )
                return S

            def emit_exp(u, S):
                """ACT: E = exp(scale*C - beta*c0), trimmed to l."""
                c = u // 4
                l = L2G[c]
                E = epool.tile([128, 2, 4, 128], BF16, tag="E", name=f"E{u}")
                nc.scalar.activation(
                    E[:, :, :, 0:l], S[:, :, :, 0:l], AF.Exp,
                    bias=expbias, scale=exp_scale,
                )
                return E

            def emit_sum(u, E):
                """PE: SumE row 8c+a += onehot64^T @ E.  Rows live in two
                64-row halves (legal matmul base partitions 0/64); each
                half is one PSUM accumulation chain over its 64 writes."""
                c, ap_ = u // 4, (u % 4) * 2
                l = L2G[c]
                sv = sumE[:].rearrange("p (g t) -> p g t", g=4)
                for j in range(2):
                    a = ap_ + j
                    r = 8 * c + a
                    h, m = r // 64, r % 64
                    nc.tensor.matmul(
                        sv[64 * h : 64 * (h + 1), :, 0:l],
                        oh64[:, 64 - m : 128 - m],
                        E[:, j, :, 0:l],
                        start=(m == 0),
                        stop=(m == 63),
                    )

            def emit_red(u, E):
                """sim1 maxE: one tensor_tensor max level (bf16 2x_1p)
                halves the elements the 1x-only reduce_max must stream."""
                c, ap_ = u // 4, (u % 4) * 2
                l = L2G[c]
                dst = sim1st[:, ap_ : ap_ + 2, 4 * c : 4 * c + 4]
                h = l // 2
                H = hpool.tile([128, 2, 4, 64], BF16, tag="H", name=f"H{u}")
                nc.vector.tensor_max(
                    H[:, :, :, 0:h], E[:, :, :, 0:h], E[:, :, :, h:l]
                )
                nc.vector.reduce_max(dst, H[:, :, :, 0:h], axis=AX.X)

            # software pipeline: PE mms for unit u, then unit u-DEFER's
            # E-consumers (keeps the PE from stalling on ACT).
            pending = []
            for u in range(NCH * 4):
                S = emit_mm(u)
                E = emit_exp(u, S)
                pending.append((u, E))
                if len(pending) > DEFER:
                    pu, pE = pending.pop(0)
                    emit_sum(pu, pE)
                    emit_red(pu, pE)
            for pu, pE in pending:
                emit_sum(pu, pE)
                emit_red(pu, pE)

            # ---- tails ----
            # The ACT Ln table saturates on inputs this small (~e^-70), so
            # take logs from the float exponent bits instead: int-convert
            # the raw bits; bits/2^k - 126.96 ~ log2(x) to +-0.03 ln after
            # the host applies the affine (exact for the w-means since
            # sum(w) = 1/2 per row).
            lnS = coll_pool.tile([128, 512], F32, tag="lnS")
            nc.vector.tensor_copy(lnS[:], sumE[:].bitcast(mybir.dt.uint32))
            m2t = coll_pool.tile([128, 512], F32, tag="m2t")
            nc.vector.tensor_mul(m2t[:], lnS[:], w2pat[:])
            m2col = coll_pool.tile([128, 4], F32, tag="m2col")
            nc.vector.tensor_reduce(
                m2col[:],
                m2t[:].rearrange("p (g t) -> p g t", g=4),
                axis=AX.X,
                op=ALU.add,
            )
            nc.sync.dma_start(m2o.ap(), m2col[:])

            # m1 (exact max): bit-log of maxE -> w1-weighted matmul per a.
            # full-w1T stationary writes an [8, 64] block per a at column
            # 64a; only row a of each block is wanted — host extracts the
            # diagonal band.
            ln1 = coll_pool.tile([128, A, B2], F32, tag="ln1")
            nc.vector.tensor_copy(
                ln1[:], sim1st[:].bitcast(mybir.dt.uint16)
            )
            m1ps = psM.tile([A, A * B2], F32, tag="m1ps")
            for a in range(A):
                nc.tensor.matmul(
                    m1ps[:, a * B2 : (a + 1) * B2],
                    w1T[:],
                    ln1[:, a, :],
                    start=True,
                    stop=True,
                )
            m1s = coll_pool.tile([A, A * B2], F32, tag="m1s")
            nc.scalar.copy(m1s[:], m1ps[:])
            nc.sync.dma_start(m1o.ap(), m1s[:])
    nc.finalize()
    return nc


def _prep(x1, mask1, x2, mask2):
    """Host-side marshaling: normalize, mask-zero, sort b, quantize."""
    x1 = np.asarray(x1, dtype=np.float32)
    x2 = np.asarray(x2, dtype=np.float32)
    m1 = np.asarray(mask1).astype(bool)
    m2 = np.asarray(mask2).astype(bool)

    EPS = 1e-8
    n1 = np.sqrt((x1 * x1).sum(-1, keepdims=True))
    n2 = np.sqrt((x2 * x2).sum(-1, keepdims=True))
    x1n = (x1 / np.maximum(n1, EPS)) * QSCALE
    x2n = (x2 / np.maximum(n2, EPS)) * QSCALE
    x1n[~m1] = 0.0
    x2n[~m2] = 0.0

    len1 = m1.sum(axis=1).astype(np.int64)
    len2 = m2.sum(axis=1).astype(np.int64)
    ext2 = np.where(m2.any(1), S2 - np.argmax(m2[:, ::-1], axis=1), 1)
    b_order = np.argsort(-ext2, kind="stable")

    def _ev(v):
        v = int(max(v, 4))
        return (v + 3) // 4 * 4   # /2-able and 4B-aligned halves (2x_1p)

    L2G = tuple(_ev(ext2[b_order[4 * c]]) for c in range(NCH))

    w1 = m1.astype(np.float32) * (0.5 / np.maximum(len1, 1))[:, None]
    w2 = m2.astype(np.float32) * (0.5 / np.maximum(len2, 1))[:, None]
    w2s = w2[b_order]                                 # [64 sorted b, 128 t]

    x2T = np.ascontiguousarray(x2n[b_order].reshape(X2W, D).T).astype(E4NP)
    w2pat = np.zeros((128, 512), np.float32)
    for c in range(NCH):
        for a in range(A):
            w2pat[8 * c + a] = w2s[4 * c : 4 * c + 4].reshape(512)

    in_maps = []
    for k in range(NCORES):
        rows = slice(k * A, (k + 1) * A)
        x1T = np.ascontiguousarray(
            x1n[rows].reshape(X1W, D).T
        ).astype(E4NP)
        consts = np.zeros((128, 4), np.float32)
        consts[:, 0] = 1.0
        consts[:, 1] = -BETA * C0
        consts[:, 2] = LNEPS
        w1Tc = np.ascontiguousarray(w1[rows].T)       # [128 s, 8 a]
        in_maps.append(
            {
                "x1p": x1T,
                "x2p": x2T,
                "consts": consts,
                "w2patp": w2pat,
                "w1Tp": w1Tc,
            }
        )
    return in_maps, b_order, L2G


def kernel(x1, mask1, x2, mask2):
    in_maps, b_order, key = _prep(x1, mask1, x2, mask2)
    if _CACHE.get("key") != key:
        _CACHE["nc"] = _build(key)
        _CACHE["key"] = key
    nc = _CACHE["nc"]
    res = run_bass_kernel_spmd(nc, in_maps, list(range(NCORES)))
    outp = np.zeros((B1, B2), dtype=np.float32)
    for k in range(NCORES):
        m1b = res.results[k]["m1o"].reshape(A, A, B2)  # [row, a-block, b]
        m2v = res.results[k]["m2o"]                   # [128 (c,a), 4]
        m1v = np.ascontiguousarray(
            m1b[np.arange(A), np.arange(A)]           # diagonal band
        )
        M2 = np.zeros((A, B2), np.float32)
        for c in range(NCH):
            for a in range(A):
                M2[a, 4 * c : 4 * c + 4] = m2v[8 * c + a]
        # bit-log affine: M held sum(w * bits); log2(x) ~ bits/2^k - 126.96
        LN2 = float(np.log(2.0))
        m1t = LN2 * (m1v / 128.0 - 126.9565 * 0.5)
        m2t = LN2 * (M2 / 8388608.0 - 126.9565 * 0.5)
        vals = (m1t + m2t) / BETA + C0
        outp[np.ix_(range(k * A, (k + 1) * A), b_order)] = vals
    return np.ascontiguousarray(outp)


# revision 34
# speedup vs baseline: 1.0799x; 1.0799x over previous
"""Trainium2 Bass kernel for DensePairwiseRelaxedWordMoverSimilarity.

Shapes (hardcoded): x1 (64,128,512) f32, mask1 (64,128) bool,
                    x2 (64,128,512) f32, mask2 (64,128) bool -> out (64,64) f32.

Strategy: data-parallel over B1 across 8 cores; core k holds x1 rows
[8k, 8k+8) plus the full x2 and produces an (8, 64) output slab.

v3: single-orientation fp8 matmul + exp-domain reductions.
For each (a, 4-b chunk) the PE computes C^T tiles [128 s, 4b x 128 t]
(fp8 DoubleRow, f32 PSUM).  The scalar engine applies
E = exp(beta*(C - c0)) into bf16 SBUF (trimmed to the chunk's baked
t-extent).  Then BOTH reductions come cheap:
  - sim1[a,b,s] = max_t C = (ln max_t E)/beta + c0: segmented DVE
    reduce_max over the free dim (optionally gpsimd pre-halving),
  - sim2[a,b,t] via log-sum-exp: sum_s E is a PE matmul with an
    all-ones stationary column, accumulated per (chunk, a) into one
    PSUM bank row; masked-s rows contribute exp(-beta*c0) ~ 1e-38 (b
    columns are sorted+trimmed so masked-t never pollutes sums).
    The LSE overshoot at beta=250 measures ~3e-3 rel on this data,
    under the fp8 quantization floor.
One Ln pass per collection (+1e-36 bias so memset-zero pad columns
stay finite) and tiny w1/w2-weighted matmul/reduce means finish on
device; host adds the (m1+m2)/beta + c0 affine and unsorts b.
"""

import numpy as np
import ml_dtypes

import concourse.bacc as bacc
import concourse.mybir as mybir
from concourse import tile
from concourse.bass_utils import run_bass_kernel_spmd

F32 = mybir.dt.float32
BF16 = mybir.dt.bfloat16
FP8 = mybir.dt.float8e4
AX = mybir.AxisListType
AF = mybir.ActivationFunctionType
ALU = mybir.AluOpType
DR = mybir.MatmulPerfMode.DoubleRow

E4NP = ml_dtypes.float8_e4m3
BFNP = ml_dtypes.bfloat16

B1, S1, B2, S2, D = 64, 128, 64, 128, 512
NCORES = 8
A = B1 // NCORES          # 8 x1 rows per core
X1W = A * S1              # 1024 x1 columns
X2W = B2 * S2             # 8192 x2 columns
NCH = 16                  # chunks of 4 sorted b's (512 x2 columns)
QSCALE = 32.0             # fp8 pre-scale; sims carry QSCALE^2
BETA = 250.0              # LSE sharpness
C0 = 0.35                 # exp recentering; keeps E <= ~1
LNEPS = 1e-36             # (unused since bit-log; kept in consts layout)
DEFER = 3                 # units of lag for E consumers (PE slack)

_CACHE = {}


def _build(L2G):
    """L2G: 16 baked t-extents (even), one per sorted 4-b chunk."""
    nc = bacc.Bacc(None, target_bir_lowering=False, debug=False)

    x1p = nc.declare_dram_parameter("x1p", [D, X1W], FP8, isOutput=False)
    x2p = nc.declare_dram_parameter("x2p", [D, X2W], FP8, isOutput=False)
    consts = nc.declare_dram_parameter("consts", [128, 4], F32, isOutput=False)
    w2patp = nc.declare_dram_parameter("w2patp", [128, 512], F32, isOutput=False)
    w1Tp = nc.declare_dram_parameter("w1Tp", [128, A], F32, isOutput=False)
    m1o = nc.declare_dram_parameter("m1o", [A, A * B2], F32, isOutput=True)
    m2o = nc.declare_dram_parameter("m2o", [128, 4], F32, isOutput=True)

    exp_scale = float(BETA / (QSCALE * QSCALE))

    with tile.TileContext(nc) as tc:
        with (
            tc.tile_pool(name="xts", bufs=1) as xts_pool,
            tc.tile_pool(name="cst", bufs=1) as cst_pool,
            tc.tile_pool(name="coll", bufs=1) as coll_pool,
            tc.tile_pool(name="epool", bufs=6) as epool,
            tc.tile_pool(name="hpool", bufs=3) as hpool,
            tc.tile_pool(name="psS", bufs=3, space="PSUM") as psS,
            tc.tile_pool(name="psF", bufs=1, space="PSUM") as psF,
            tc.tile_pool(name="psM", bufs=1, space="PSUM") as psM,
        ):
            # ---- loads: first chunk + x1 first (they gate the first
            # matmul), then constants, then the remaining x2 blocks.
            x2c = [None] * NCH
            blocks = [(0, 1), (1, 1), (2, 2), (4, 4), (8, 8)]
            xb0 = xts_pool.tile([128, 4, 512], FP8, tag="xb0")
            nc.sync.dma_start(
                xb0[:],
                x2p.ap()[:, 0:512].rearrange("(k p) m -> p k m", p=128),
            )
            x2c[0] = xb0[:, :, :]
            # x1 lands as four a-pair tiles so unit 0 can start after the
            # first one (tile-granular dependency tracking)
            x1q = []
            for q in range(4):
                xq = xts_pool.tile([128, 4, 256], FP8, tag=f"x1q{q}")
                nc.sync.dma_start(
                    xq[:],
                    x1p.ap()[:, q * 256 : (q + 1) * 256].rearrange(
                        "(k p) m -> p k m", p=128
                    ),
                )
                x1q.append(xq)
            csts = cst_pool.tile([128, 4], F32, tag="consts")
            nc.sync.dma_start(csts[:], consts.ap())
            for g0, w in blocks[1:]:
                xb = xts_pool.tile([128, 4, w * 512], FP8, tag=f"xb{g0}")
                nc.sync.dma_start(
                    xb[:],
                    x2p.ap()[:, g0 * 512 : (g0 + w) * 512].rearrange(
                        "(k p) m -> p k m", p=128
                    ),
                )
                for j in range(w):
                    x2c[g0 + j] = xb[:, :, j * 512 : (j + 1) * 512]
            w2pat = cst_pool.tile([128, 512], F32, tag="w2pat")
            nc.sync.dma_start(w2pat[:], w2patp.ap())
            w1T = cst_pool.tile([128, A], F32, tag="w1T")
            nc.sync.dma_start(w1T[:], w1Tp.ap())

            expbias = csts[:, 1:2]      # -BETA*C0
            # sliding-window one-hot: col 64 is all-ones, so the width-64
            # slice [64-m : 128-m] has its ones in column m.
            oh64 = cst_pool.tile([128, 128], BF16, tag="oh64")
            nc.vector.memset(oh64[:], 0.0)
            nc.vector.tensor_copy(
                oh64[:, 64:65], csts[:, 0:1]
            )

            # maxE collection [s, a, sorted-b]; SumE PSUM bank rows (8c+a)
            sim1st = coll_pool.tile([128, A, B2], BF16, tag="sim1st")
            sumE = psF.tile([128, 512], F32, tag="sumE")
            nc.vector.memset(sumE[:], 0.0)

            def emit_mm(u):
                """fp8 DoubleRow matmuls for unit u = (chunk, a-pair).
                Full 512-col moving: per-column trimming loses more to
                fixed per-instruction overhead than it saves."""
                c, ap_ = u // 4, (u % 4) * 2
                S = psS.tile([128, 2, 4, 128], F32, tag="S", name=f"S{u}")
                Sf = S[:].rearrange("p j g t -> p j (g t)")
                for j in range(2):
                    a = ap_ + j
                    for i, (k0, k1) in enumerate(((0, 2), (2, 4))):
                        nc.tensor.matmul(
                            Sf[:, j, :],
                            x1q[a // 2][:, k0:k1, (a % 2) * 128 : (a % 2) * 128 + 128],
                            x2c[c][:, k0:k1, :],
                            start=(i == 0),
                            stop=(i == 1),
                            perf_mode=DR,
                        )
                return S

            def emit_exp(u, S):
                """ACT: E = exp(scale*C - beta*c0), trimmed to l."""
                c = u // 4
                l = L2G[c]
                E = epool.tile([128, 2, 4, 128], BF16, tag="E", name=f"E{u}")
                nc.scalar.activation(
                    E[:, :, :, 0:l], S[:, :, :, 0:l], AF.Exp,
                    bias=expbias, scale=exp_scale,
                )
                return E

            def emit_sum(u, E):
                """PE: SumE row 8c+a += onehot64^T @ E.  Rows live in two
                64-row halves (legal matmul base partitions 0/64); each
                half is one PSUM accumulation chain over its 64 writes."""
                c, ap_ = u // 4, (u % 4) * 2
                l = L2G[c]
                sv = sumE[:].rearrange("p (g t) -> p g t", g=4)
                for j in range(2):
                    a = ap_ + j
                    r = 8 * c + a
                    h, m = r // 64, r % 64
                    nc.tensor.matmul(
                        sv[64 * h : 64 * (h + 1), :, 0:l],
                        oh64[:, 64 - m : 128 - m],
                        E[:, j, :, 0:l],
                        start=(m == 0),
                        stop=(m == 63),
                    )

            def emit_red(u, E):
                """sim1 maxE: one tensor_tensor max level (bf16 2x_1p)
                halves the elements the 1x-only reduce_max must stream."""
                c, ap_ = u // 4, (u % 4) * 2
                l = L2G[c]
                dst = sim1st[:, ap_ : ap_ + 2, 4 * c : 4 * c + 4]
                h = l // 2
                H = hpool.tile([128, 2, 4, 64], BF16, tag="H", name=f"H{u}")
                nc.vector.tensor_max(
                    H[:, :, :, 0:h], E[:, :, :, 0:h], E[:, :, :, h:l]
                )
                nc.vector.reduce_max(dst, H[:, :, :, 0:h], axis=AX.X)

            # ---- finale (emitted per 64-row half as its chains close) ----
            # The ACT Ln table saturates on inputs this small (~e^-70), so
            # take logs from the float exponent bits instead: int-convert
            # the raw bits; bits/2^k - 126.96 ~ log2(x) to +-0.03 ln after
            # the host applies the affine (exact for the w-means since
            # sum(w) = 1/2 per row).
            lnS = coll_pool.tile([128, 512], F32, tag="lnS")
            m2t = coll_pool.tile([128, 512], F32, tag="m2t")
            m2col = coll_pool.tile([128, 4], F32, tag="m2col")
            ln1 = coll_pool.tile([128, A, B2], F32, tag="ln1")
            m1ps = psM.tile([A, A * B2], F32, tag="m1ps")

            def emit_finale(h):
                """h-th 64-row half of SumE / 32-col half of sim1st."""
                p0, p1 = 64 * h, 64 * (h + 1)
                b0, b1 = 32 * h, 32 * (h + 1)
                nc.vector.tensor_copy(
                    lnS[p0:p1, :], sumE[p0:p1, :].bitcast(mybir.dt.uint32)
                )
                nc.vector.tensor_mul(
                    m2t[p0:p1, :], lnS[p0:p1, :], w2pat[p0:p1, :]
                )
                nc.vector.tensor_reduce(
                    m2col[p0:p1, :],
                    m2t[p0:p1, :].rearrange("p (g t) -> p g t", g=4),
                    axis=AX.X,
                    op=ALU.add,
                )
                nc.sync.dma_start(m2o.ap()[p0:p1, :], m2col[p0:p1, :])
                # m1: bit-log of maxE -> w1-weighted matmul per a; full-w1T
                # stationary writes an [8, 32] block per a at column
                # 64a + b0; host extracts the diagonal band.
                nc.vector.tensor_copy(
                    ln1[:, :, b0:b1],
                    sim1st[:, :, b0:b1].bitcast(mybir.dt.uint16),
                )
                for a in range(A):
                    nc.tensor.matmul(
                        m1ps[:, a * B2 + b0 : a * B2 + b1],
                        w1T[:],
                        ln1[:, a, b0:b1],
                        start=True,
                        stop=True,
                    )

            # software pipeline: PE mms for unit u, then unit u-DEFER's
            # E-consumers (keeps the PE from stalling on ACT).
            pending = []
            for u in range(NCH * 4):
                S = emit_mm(u)
                E = emit_exp(u, S)
                pending.append((u, E))
                if len(pending) > DEFER:
                    pu, pE = pending.pop(0)
                    emit_sum(pu, pE)
                    emit_red(pu, pE)
                    if pu == 31:
                        emit_finale(0)
            for pu, pE in pending:
                emit_sum(pu, pE)
                emit_red(pu, pE)
            emit_finale(1)

            m1s = coll_pool.tile([A, A * B2], F32, tag="m1s")
            nc.scalar.copy(m1s[:], m1ps[:])
            nc.sync.dma_start(m1o.ap(), m1s[:])
    nc.finalize()
    return nc


def _prep(x1, mask1, x2, mask2):
    """Host-side marshaling: normalize, mask-zero, sort b, quantize."""
    x1 = np.asarray(x1, dtype=np.float32)
    x2 = np.asarray(x2, dtype=np.float32)
    m1 = np.asarray(mask1).astype(bool)
    m2 = np.asarray(mask2).astype(bool)

    EPS = 1e-8
    n1 = np.sqrt((x1 * x1).sum(-1, keepdims=True))
    n2 = np.sqrt((x2 * x2).sum(-1, keepdims=True))
    x1n = (x1 / np.maximum(n1, EPS)) * QSCALE
    x2n = (x2 / np.maximum(n2, EPS)) * QSCALE
    x1n[~m1] = 0.0
    x2n[~m2] = 0.0

    len1 = m1.sum(axis=1).astype(np.int64)
    len2 = m2.sum(axis=1).astype(np.int64)
    ext2 = np.where(m2.any(1), S2 - np.argmax(m2[:, ::-1], axis=1), 1)
    b_order = np.argsort(-ext2, kind="stable")

    def _ev(v):
        v = int(max(v, 4))
        return (v + 3) // 4 * 4   # /2-able and 4B-aligned halves (2x_1p)

    L2G = tuple(_ev(ext2[b_order[4 * c]]) for c in range(NCH))

    w1 = m1.astype(np.float32) * (0.5 / np.maximum(len1, 1))[:, None]
    w2 = m2.astype(np.float32) * (0.5 / np.maximum(len2, 1))[:, None]
    w2s = w2[b_order]                                 # [64 sorted b, 128 t]

    x2T = np.ascontiguousarray(x2n[b_order].reshape(X2W, D).T).astype(E4NP)
    w2pat = np.zeros((128, 512), np.float32)
    for c in range(NCH):
        for a in range(A):
            w2pat[8 * c + a] = w2s[4 * c : 4 * c + 4].reshape(512)

    in_maps = []
    for k in range(NCORES):
        rows = slice(k * A, (k + 1) * A)
        x1T = np.ascontiguousarray(
            x1n[rows].reshape(X1W, D).T
        ).astype(E4NP)
        consts = np.zeros((128, 4), np.float32)
        consts[:, 0] = 1.0
        consts[:, 1] = -BETA * C0
        consts[:, 2] = LNEPS
        w1Tc = np.ascontiguousarray(w1[rows].T)       # [128 s, 8 a]
        in_maps.append(
            {
                "x1p": x1T,
                "x2p": x2T,
                "consts": consts,
                "w2patp": w2pat,
                "w1Tp": w1Tc,
            }
        )
    return in_maps, b_order, L2G


def kernel(x1, mask1, x2, mask2):
    in_maps, b_order, key = _prep(x1, mask1, x2, mask2)
    if _CACHE.get("key") != key:
        _CACHE["nc"] = _build(key)
        _CACHE["key"] = key
    nc = _CACHE["nc"]
    res = run_bass_kernel_spmd(nc, in_maps, list(range(NCORES)))
    outp = np.zeros((B1, B2), dtype=np.float32)
    for k in range(NCORES):
        m1b = res.results[k]["m1o"].reshape(A, A, B2)  # [row, a-block, b]
        m2v = res.results[k]["m2o"]                   # [128 (c,a), 4]
        m1v = np.ascontiguousarray(
            m1b[np.arange(A), np.arange(A)]           # diagonal band
        )
        M2 = np.zeros((A, B2), np.float32)
        for c in range(NCH):
            for a in range(A):
                M2[a, 4 * c : 4 * c + 4] = m2v[8 * c + a]
        # bit-log affine: M held sum(w * bits); log2(x) ~ bits/2^k - 126.96
        LN2 = float(np.log(2.0))
        m1t = LN2 * (m1v / 128.0 - 126.9565 * 0.5)
        m2t = LN2 * (M2 / 8388608.0 - 126.9565 * 0.5)
        vals = (m1t + m2t) / BETA + C0
        outp[np.ix_(range(k * A, (k + 1) * A), b_order)] = vals
    return np.ascontiguousarray(outp)


# revision 39
# speedup vs baseline: 1.2024x; 1.1134x over previous
"""Trainium2 Bass kernel for DensePairwiseRelaxedWordMoverSimilarity.

Shapes (hardcoded): x1 (64,128,512) f32, mask1 (64,128) bool,
                    x2 (64,128,512) f32, mask2 (64,128) bool -> out (64,64) f32.

Strategy: data-parallel over B1 across 8 cores; core k holds x1 rows
[8k, 8k+8) plus the full x2 and produces an (8, 64) output slab.

v3: single-orientation fp8 matmul + exp-domain reductions.
For each (a, 4-b chunk) the PE computes C^T tiles [128 s, 4b x 128 t]
(fp8 DoubleRow, f32 PSUM).  The scalar engine applies
E = exp(beta*(C - c0)) into bf16 SBUF (trimmed to the chunk's baked
t-extent).  Then BOTH reductions come cheap:
  - sim1[a,b,s] = max_t C = (ln max_t E)/beta + c0: segmented DVE
    reduce_max over the free dim (optionally gpsimd pre-halving),
  - sim2[a,b,t] via log-sum-exp: sum_s E is a PE matmul with an
    all-ones stationary column, accumulated per (chunk, a) into one
    PSUM bank row; masked-s rows contribute exp(-beta*c0) ~ 1e-38 (b
    columns are sorted+trimmed so masked-t never pollutes sums).
    The LSE overshoot at beta=250 measures ~3e-3 rel on this data,
    under the fp8 quantization floor.
One Ln pass per collection (+1e-36 bias so memset-zero pad columns
stay finite) and tiny w1/w2-weighted matmul/reduce means finish on
device; host adds the (m1+m2)/beta + c0 affine and unsorts b.
"""

import numpy as np
import ml_dtypes

import concourse.bacc as bacc
import concourse.mybir as mybir
from concourse import tile
from concourse.bass_utils import run_bass_kernel_spmd

F32 = mybir.dt.float32
BF16 = mybir.dt.bfloat16
FP8 = mybir.dt.float8e4
AX = mybir.AxisListType
AF = mybir.ActivationFunctionType
ALU = mybir.AluOpType
DR = mybir.MatmulPerfMode.DoubleRow

E4NP = ml_dtypes.float8_e4m3
BFNP = ml_dtypes.bfloat16

B1, S1, B2, S2, D = 64, 128, 64, 128, 512
NCORES = 8
A = B1 // NCORES          # 8 x1 rows per core
X1W = A * S1              # 1024 x1 columns
X2W = B2 * S2             # 8192 x2 columns
NCH = 16                  # chunks of 4 sorted b's (512 x2 columns)
QSCALE = 32.0             # fp8 pre-scale; sims carry QSCALE^2
BETA = 250.0              # LSE sharpness
C0 = 0.35                 # exp recentering; keeps E <= ~1
LNEPS = 1e-36             # (unused since bit-log; kept in consts layout)
DEFER = 3                 # units of lag for E consumers (PE slack)

_CACHE = {}


def _build(L2G):
    """L2G: 16 baked t-extents (mult of 4), one per sorted 4-b chunk.
    x2 arrives host-packed: chunk c's 4 b-columns sit back-to-back at
    stride L2G[c] starting at byte offset OFF[c], so the DR matmuls
    stream only ~78% of the columns at no extra instruction cost."""
    OFF = [0]
    for c in range(NCH):
        OFF.append(OFF[-1] + 4 * L2G[c])
    W = OFF[NCH]
    nc = bacc.Bacc(None, target_bir_lowering=False, debug=False)

    x1p = nc.declare_dram_parameter("x1p", [D, X1W], FP8, isOutput=False)
    x2p = nc.declare_dram_parameter("x2p", [D, W], FP8, isOutput=False)
    consts = nc.declare_dram_parameter("consts", [128, 4], F32, isOutput=False)
    w2patp = nc.declare_dram_parameter("w2patp", [128, 512], F32, isOutput=False)
    w1Tp = nc.declare_dram_parameter("w1Tp", [128, A], F32, isOutput=False)
    m1o = nc.declare_dram_parameter("m1o", [A, A * B2], F32, isOutput=True)
    m2o = nc.declare_dram_parameter("m2o", [128, 4], F32, isOutput=True)

    exp_scale = float(BETA / (QSCALE * QSCALE))

    with tile.TileContext(nc) as tc:
        with (
            tc.tile_pool(name="xts", bufs=1) as xts_pool,
            tc.tile_pool(name="cst", bufs=1) as cst_pool,
            tc.tile_pool(name="coll", bufs=1) as coll_pool,
            tc.tile_pool(name="epool", bufs=6) as epool,
            tc.tile_pool(name="hpool", bufs=3) as hpool,
            tc.tile_pool(name="psS", bufs=3, space="PSUM") as psS,
            tc.tile_pool(name="psF", bufs=1, space="PSUM") as psF,
            tc.tile_pool(name="psM", bufs=1, space="PSUM") as psM,
        ):
            # ---- loads: first chunk + x1 first (they gate the first
            # matmul), then constants, then the remaining x2 blocks.
            x2c = [None] * NCH
            blocks = [(0, 1), (1, 1), (2, 2), (4, 4), (8, 8)]
            xb0 = xts_pool.tile([128, 4, 4 * L2G[0]], FP8, tag="xb0")
            nc.sync.dma_start(
                xb0[:],
                x2p.ap()[:, OFF[0] : OFF[1]].rearrange(
                    "(k p) m -> p k m", p=128
                ),
            )
            x2c[0] = xb0[:, :, :]
            # x1 lands as four a-pair tiles so unit 0 can start after the
            # first one (tile-granular dependency tracking)
            x1q = []
            for q in range(4):
                xq = xts_pool.tile([128, 4, 256], FP8, tag=f"x1q{q}")
                nc.sync.dma_start(
                    xq[:],
                    x1p.ap()[:, q * 256 : (q + 1) * 256].rearrange(
                        "(k p) m -> p k m", p=128
                    ),
                )
                x1q.append(xq)
            csts = cst_pool.tile([128, 4], F32, tag="consts")
            nc.sync.dma_start(csts[:], consts.ap())
            for g0, w in blocks[1:]:
                wcols = OFF[g0 + w] - OFF[g0]
                xb = xts_pool.tile([128, 4, wcols], FP8, tag=f"xb{g0}")
                nc.sync.dma_start(
                    xb[:],
                    x2p.ap()[:, OFF[g0] : OFF[g0 + w]].rearrange(
                        "(k p) m -> p k m", p=128
                    ),
                )
                for j in range(w):
                    o0 = OFF[g0 + j] - OFF[g0]
                    o1 = OFF[g0 + j + 1] - OFF[g0]
                    x2c[g0 + j] = xb[:, :, o0:o1]
            w2pat = cst_pool.tile([128, 512], F32, tag="w2pat")
            nc.sync.dma_start(w2pat[:], w2patp.ap())
            w1T = cst_pool.tile([128, A], F32, tag="w1T")
            nc.sync.dma_start(w1T[:], w1Tp.ap())

            expbias = csts[:, 1:2]      # -BETA*C0
            # sliding-window one-hot: col 64 is all-ones, so the width-64
            # slice [64-m : 128-m] has its ones in column m.
            oh64 = cst_pool.tile([128, 128], BF16, tag="oh64")
            nc.vector.memset(oh64[:], 0.0)
            nc.vector.tensor_copy(
                oh64[:, 64:65], csts[:, 0:1]
            )

            # maxE collection [s, a, sorted-b]; SumE PSUM bank rows (8c+a)
            sim1st = coll_pool.tile([128, A, B2], BF16, tag="sim1st")
            sumE = psF.tile([128, 512], F32, tag="sumE")
            nc.vector.memset(sumE[:], 0.0)

            def emit_mm(u):
                """fp8 DoubleRow matmuls for unit u = (chunk, a-pair);
                the host-packed moving streams only valid columns."""
                c, ap_ = u // 4, (u % 4) * 2
                w = 4 * L2G[c]
                S = psS.tile([128, 2, 512], F32, tag="S", name=f"S{u}")
                for j in range(2):
                    a = ap_ + j
                    for i, (k0, k1) in enumerate(((0, 2), (2, 4))):
                        nc.tensor.matmul(
                            S[:, j, 0:w],
                            x1q[a // 2][:, k0:k1, (a % 2) * 128 : (a % 2) * 128 + 128],
                            x2c[c][:, k0:k1, :],
                            start=(i == 0),
                            stop=(i == 1),
                            perf_mode=DR,
                        )
                return S

            def emit_exp(u, S):
                """ACT: E = exp(scale*C - beta*c0) over the packed cols."""
                c = u // 4
                w = 4 * L2G[c]
                E = epool.tile([128, 2, 512], BF16, tag="E", name=f"E{u}")
                nc.scalar.activation(
                    E[:, :, 0:w], S[:, :, 0:w], AF.Exp,
                    bias=expbias, scale=exp_scale,
                )
                return E

            def emit_sum(u, E):
                """PE: SumE row 8c+a += onehot64^T @ E.  Rows live in two
                64-row halves (legal matmul base partitions 0/64); each
                half is one PSUM accumulation chain over its 64 writes.
                The strided output AP re-spreads the packed columns onto
                uniform 128-stride segments."""
                c, ap_ = u // 4, (u % 4) * 2
                l = L2G[c]
                sv = sumE[:].rearrange("p (g t) -> p g t", g=4)
                for j in range(2):
                    a = ap_ + j
                    r = 8 * c + a
                    h, m = r // 64, r % 64
                    nc.tensor.matmul(
                        sv[64 * h : 64 * (h + 1), :, 0:l],
                        oh64[:, 64 - m : 128 - m],
                        E[:, j, 0 : 4 * l],
                        start=(m == 0),
                        stop=(m == 63),
                    )

            def emit_red(u, E):
                """sim1 maxE: one tensor_tensor max level (bf16 2x_1p)
                halves the elements the 1x-only reduce_max must stream."""
                c, ap_ = u // 4, (u % 4) * 2
                l = L2G[c]
                dst = sim1st[:, ap_ : ap_ + 2, 4 * c : 4 * c + 4]
                h = l // 2
                Ev = E[:, :, 0 : 4 * l].rearrange("p j (g t) -> p j g t", g=4)
                H = hpool.tile([128, 2, 4, 64], BF16, tag="H", name=f"H{u}")
                nc.vector.tensor_max(
                    H[:, :, :, 0:h], Ev[:, :, :, 0:h], Ev[:, :, :, h:l]
                )
                nc.vector.reduce_max(dst, H[:, :, :, 0:h], axis=AX.X)

            # ---- finale (emitted per 64-row half as its chains close) ----
            # The ACT Ln table saturates on inputs this small (~e^-70), so
            # take logs from the float exponent bits instead: int-convert
            # the raw bits; bits/2^k - 126.96 ~ log2(x) to +-0.03 ln after
            # the host applies the affine (exact for the w-means since
            # sum(w) = 1/2 per row).
            lnS = coll_pool.tile([128, 512], F32, tag="lnS")
            m2t = coll_pool.tile([128, 512], F32, tag="m2t")
            m2col = coll_pool.tile([128, 4], F32, tag="m2col")
            ln1 = coll_pool.tile([128, A, B2], F32, tag="ln1")
            m1ps = psM.tile([A, A * B2], F32, tag="m1ps")

            def emit_finale(h):
                """h-th 64-row half of SumE / 32-col half of sim1st."""
                p0, p1 = 64 * h, 64 * (h + 1)
                b0, b1 = 32 * h, 32 * (h + 1)
                nc.vector.tensor_copy(
                    lnS[p0:p1, :], sumE[p0:p1, :].bitcast(mybir.dt.uint32)
                )
                nc.vector.tensor_mul(
                    m2t[p0:p1, :], lnS[p0:p1, :], w2pat[p0:p1, :]
                )
                nc.vector.tensor_reduce(
                    m2col[p0:p1, :],
                    m2t[p0:p1, :].rearrange("p (g t) -> p g t", g=4),
                    axis=AX.X,
                    op=ALU.add,
                )
                nc.sync.dma_start(m2o.ap()[p0:p1, :], m2col[p0:p1, :])
                # m1: bit-log of maxE -> w1-weighted matmul per a; full-w1T
                # stationary writes an [8, 32] block per a at column
                # 64a + b0; host extracts the diagonal band.
                nc.vector.tensor_copy(
                    ln1[:, :, b0:b1],
                    sim1st[:, :, b0:b1].bitcast(mybir.dt.uint16),
                )
                for a in range(A):
                    nc.tensor.matmul(
                        m1ps[:, a * B2 + b0 : a * B2 + b1],
                        w1T[:],
                        ln1[:, a, b0:b1],
                        start=True,
                        stop=True,
                    )

            # software pipeline: PE mms for unit u, then unit u-DEFER's
            # E-consumers (keeps the PE from stalling on ACT).
            pending = []
            for u in range(NCH * 4):
                S = emit_mm(u)
                E = emit_exp(u, S)
                pending.append((u, E))
                if len(pending) > DEFER:
                    pu, pE = pending.pop(0)
                    emit_sum(pu, pE)
                    emit_red(pu, pE)
                    if pu == 31:
                        emit_finale(0)
            for pu, pE in pending:
                emit_sum(pu, pE)
                emit_red(pu, pE)
            emit_finale(1)

            m1s = coll_pool.tile([A, A * B2], F32, tag="m1s")
            nc.scalar.copy(m1s[:], m1ps[:])
            nc.sync.dma_start(m1o.ap(), m1s[:])
    nc.finalize()
    return nc


def _prep(x1, mask1, x2, mask2):
    """Host-side marshaling: normalize, mask-zero, sort b, quantize."""
    x1 = np.asarray(x1, dtype=np.float32)
    x2 = np.asarray(x2, dtype=np.float32)
    m1 = np.asarray(mask1).astype(bool)
    m2 = np.asarray(mask2).astype(bool)

    EPS = 1e-8
    n1 = np.sqrt((x1 * x1).sum(-1, keepdims=True))
    n2 = np.sqrt((x2 * x2).sum(-1, keepdims=True))
    x1n = (x1 / np.maximum(n1, EPS)) * QSCALE
    x2n = (x2 / np.maximum(n2, EPS)) * QSCALE
    x1n[~m1] = 0.0
    x2n[~m2] = 0.0

    len1 = m1.sum(axis=1).astype(np.int64)
    len2 = m2.sum(axis=1).astype(np.int64)
    ext2 = np.where(m2.any(1), S2 - np.argmax(m2[:, ::-1], axis=1), 1)
    b_order = np.argsort(-ext2, kind="stable")

    def _ev(v):
        v = int(max(v, 4))
        return (v + 3) // 4 * 4   # /2-able and 4B-aligned halves (2x_1p)

    L2G = tuple(_ev(ext2[b_order[4 * c]]) for c in range(NCH))

    w1 = m1.astype(np.float32) * (0.5 / np.maximum(len1, 1))[:, None]
    w2 = m2.astype(np.float32) * (0.5 / np.maximum(len2, 1))[:, None]
    w2s = w2[b_order]                                 # [64 sorted b, 128 t]

    # pack x2 columns: chunk c's 4 b's back-to-back at stride L2G[c]
    x2s = x2n[b_order]                                # [64, 128, 512]
    Wp = 4 * sum(L2G)
    x2pk = np.zeros((Wp, D), np.float32)
    off = 0
    for c in range(NCH):
        l = L2G[c]
        for g in range(4):
            x2pk[off : off + min(l, S2)] = x2s[4 * c + g, :l]
            off += l
    x2T = np.ascontiguousarray(x2pk.T).astype(E4NP)
    w2pat = np.zeros((128, 512), np.float32)
    for c in range(NCH):
        for a in range(A):
            w2pat[8 * c + a] = w2s[4 * c : 4 * c + 4].reshape(512)

    in_maps = []
    for k in range(NCORES):
        rows = slice(k * A, (k + 1) * A)
        x1T = np.ascontiguousarray(
            x1n[rows].reshape(X1W, D).T
        ).astype(E4NP)
        consts = np.zeros((128, 4), np.float32)
        consts[:, 0] = 1.0
        consts[:, 1] = -BETA * C0
        consts[:, 2] = LNEPS
        w1Tc = np.ascontiguousarray(w1[rows].T)       # [128 s, 8 a]
        in_maps.append(
            {
                "x1p": x1T,
                "x2p": x2T,
                "consts": consts,
                "w2patp": w2pat,
                "w1Tp": w1Tc,
            }
        )
    return in_maps, b_order, L2G


def kernel(x1, mask1, x2, mask2):
    in_maps, b_order, key = _prep(x1, mask1, x2, mask2)
    if _CACHE.get("key") != key:
        _CACHE["nc"] = _build(key)
        _CACHE["key"] = key
    nc = _CACHE["nc"]
    res = run_bass_kernel_spmd(nc, in_maps, list(range(NCORES)))
    outp = np.zeros((B1, B2), dtype=np.float32)
    for k in range(NCORES):
        m1b = res.results[k]["m1o"].reshape(A, A, B2)  # [row, a-block, b]
        m2v = res.results[k]["m2o"]                   # [128 (c,a), 4]
        m1v = np.ascontiguousarray(
            m1b[np.arange(A), np.arange(A)]           # diagonal band
        )
        M2 = np.zeros((A, B2), np.float32)
        for c in range(NCH):
            for a in range(A):
                M2[a, 4 * c : 4 * c + 4] = m2v[8 * c + a]
        # bit-log affine: M held sum(w * bits); log2(x) ~ bits/2^k - 126.96
        LN2 = float(np.log(2.0))
        m1t = LN2 * (m1v / 128.0 - 126.9565 * 0.5)
        m2t = LN2 * (M2 / 8388608.0 - 126.9565 * 0.5)
        vals = (m1t + m2t) / BETA + C0
        outp[np.ix_(range(k * A, (k + 1) * A), b_order)] = vals
    return np.ascontiguousarray(outp)
